# revision 6
# baseline (speedup 1.0000x reference)
"""Trainium2 Bass kernel for a 32-head causal attention layer.

Problem: B=1, S=2048, D=4096, 32 heads x 128 head-dim, fp32, llama-style
interleaved RoPE on q/k, KV-cache index_copy (identity for arange indexes),
additive mask + softmax, output projection.

Sharding (8 NeuronCores, tensor-parallel by heads):
  - core c owns heads [4c, 4c+4): wq/wk/wv output rows [512c, 512c+512)
  - per-core: QKV projections -> RoPE -> attention -> attn.T shard [512, 2048]
  - 4 chunked on-device AllGathers (one per 512-query block) overlap the
    collective with attention/output-projection compute
  - core c computes output column shard out[:, 512c:512c+512] = attn @ wo_c.T
  - host concatenates the 8 column shards (pure unshard, no arithmetic)

All matmul operands are bf16 (RNE-rounded on host for the inputs; on-device
producers write bf16-dtype tiles which round on writeback).  PSUM
accumulation is fp32 throughout.  bf16 halves DMA/SBUF/collective traffic,
which is what keeps the PE array fed (fp32 operand streaming saturated the
DMA engines and stalled the PE at every phase boundary).

Mask modes (selected on host per input):
  - "zero": mask is all zeros (full bidirectional attention) -- no mask
    tensor on device at all.
  - "causal": canonical tril mask -- per-128-key-block multiplicative mask
    on just the diagonal blocks; off-diagonal key blocks are skipped.
  - "generic": arbitrary additive mask streamed from DRAM.

RoPE trick: weight rows of wq/wk are permuted per head on the host so the
interleaved pairs (2j, 2j+1) become (j, j+64).  Scores are invariant under
a per-head orthogonal permutation applied to both q and k, and the rotation
then only needs partition-range [0:64]/[64:128] cross-multiplies, which map
to plain DVE tensor_tensor ops (no strided partition access).

The 1/sqrt(128) score scale is folded into the Exp activation's scale
operand.  Softmax runs over the partition (key) axis: scores are computed
transposed st[k, q] = K Q^T, summed with a ones-vector matmul, and
normalized after the PV matmul via reciprocal_approx_fast + outer-product
broadcast.
"""

import numpy as np
import ml_dtypes

import concourse.bass as bass
import concourse.mybir as mybir
import concourse.tile as tile
from concourse import bacc
from concourse.bass_utils import run_bass_kernel_spmd

F32 = mybir.dt.float32
BF16 = mybir.dt.bfloat16

S = 2048
D = 4096
HD = 128
N_HEADS = 32
N_CORES = 8
HPC = N_HEADS // N_CORES          # heads per core = 4
FC = HPC * HD                     # features per core = 512
N_DC = D // 128                   # 32 contraction chunks
N_SC = S // 512                   # 4 seq chunks of 512
N_KC = S // 128                   # 16 key chunks of 128
SCALE = 1.0 / np.sqrt(HD)
NEG = -1e9


def to_bf16(a: np.ndarray) -> np.ndarray:
    return np.ascontiguousarray(a, dtype=np.float32).astype(ml_dtypes.bfloat16)


def _build_module(mode: str):
    assert mode in ("zero", "causal", "generic")
    nc = bacc.Bacc(num_devices=N_CORES)

    xT = nc.dram_tensor("xT", [N_SC * D, 512], BF16, kind="ExternalInput")
    wqk_t = nc.dram_tensor("wqk_t", [D, 2 * FC], BF16, kind="ExternalInput")
    wv_t = nc.dram_tensor("wv_t", [D, FC], BF16, kind="ExternalInput")
    wo_t = nc.dram_tensor("wo_t", [D, FC], BF16, kind="ExternalInput")
    cosb = nc.dram_tensor("cosb", [128, S], F32, kind="ExternalInput")
    sinb = nc.dram_tensor("sinb", [128, S], F32, kind="ExternalInput")
    if mode == "causal":
        bmask = nc.dram_tensor("bmask", [128, 4 * 512], F32, kind="ExternalInput")
    elif mode == "generic":
        maskT = nc.dram_tensor("maskT", [S, S], F32, kind="ExternalInput")
    out_t = nc.dram_tensor("out", [S, FC], F32, kind="ExternalOutput")

    with tile.TileContext(nc) as tc:
        with tc.tile_pool(name="const", bufs=1) as constp, \
             tc.tile_pool(name="dram", bufs=1, space="DRAM") as dram:
            cc_in = [dram.tile([FC, 512], BF16, name=f"cc_in{i}")
                     for i in range(N_SC)]
            cc_out = [dram.tile([D, 512], BF16, addr_space="Shared",
                                name=f"cc_out{i}") for i in range(N_SC)]

            ones_f = constp.tile([128, 1], F32, tag="ones_f")
            nc.vector.memset(ones_f[:], 1.0)
            ones_col = constp.tile([128, 1], BF16, tag="ones_col")
            nc.vector.tensor_copy(ones_col[:], ones_f[:])

            with tc.tile_pool(name="qkv", bufs=1) as qkvp:
                qt = [qkvp.tile([128, S], BF16, tag=f"qt{h}", name=f"qt{h}")
                      for h in range(HPC)]
                kt = [qkvp.tile([128, S], BF16, tag=f"kt{h}", name=f"kt{h}")
                      for h in range(HPC)]
                vt = [qkvp.tile([128, FC], BF16, tag=f"vt{b}", name=f"vt{b}")
                      for b in range(N_KC)]

                # -------- phase 1: QKV projections + rope.  Three separate
                # passes (Q, K, V) so each 512-seq chunk only needs 4 PSUM
                # banks; two chunks pipeline through the 8 banks and the
                # PSUM eviction (rope / copy) of one chunk overlaps the next
                # chunk's matmuls.  x is re-streamed per pass (bf16 DMA has
                # 2x+ slack vs the PE here).
                with tc.tile_pool(name="p1sb", bufs=1) as p1sb, \
                     tc.tile_pool(name="p1ps", bufs=8, space="PSUM") as p1ps:
                    cos_sb = p1sb.tile([128, S], F32, tag="cos")
                    sin_sb = p1sb.tile([128, S], F32, tag="sin")
                    nc.sync.dma_start(cos_sb[:], cosb[:])
                    nc.sync.dma_start(sin_sb[:], sinb[:])

                    def rope_evict(ps, dst, sc, tag):
                        # dst[0:64]  = ps[0:64]*c - ps[64:128]*s
                        # dst[64:128]= ps[0:64]*s + ps[64:128]*c
                        # (cross-partition reads only on the PSUM operand:
                        # both-SBUF operands need equal base partitions per
                        # the BIR verifier; cos rows are duplicated so the
                        # cos multiply is one full-width op)
                        s0 = sc * 512
                        cs = cos_sb[:, s0:s0 + 512]
                        sn = sin_sb[:, s0:s0 + 512]
                        t1 = p1sb.tile([128, 512], F32, tag="t1", bufs=2,
                                       name=f"t1_{tag}")
                        t2 = p1sb.tile([128, 512], F32, tag="t2", bufs=2,
                                       name=f"t2_{tag}")
                        t3 = p1sb.tile([128, 512], F32, tag="t3", bufs=2,
                                       name=f"t3_{tag}")
                        mul = mybir.AluOpType.mult
                        nc.vector.tensor_tensor(out=t1[:], in0=ps[:],
                                                in1=cs, op=mul)
                        nc.vector.tensor_tensor(out=t2[0:64, :], in0=ps[64:128, :],
                                                in1=sn[0:64, :], op=mul)
                        nc.vector.tensor_tensor(out=t3[64:128, :], in0=ps[0:64, :],
                                                in1=sn[64:128, :], op=mul)
                        nc.vector.tensor_tensor(out=dst[0:64, s0:s0 + 512],
                                                in0=t1[0:64, :], in1=t2[0:64, :],
                                                op=mybir.AluOpType.subtract)
                        nc.vector.tensor_tensor(out=dst[64:128, s0:s0 + 512],
                                                in0=t3[64:128, :], in1=t1[64:128, :],
                                                op=mybir.AluOpType.add)

                    # Q pass, then K pass: stationary = weight column block,
                    # moving = x chunk; out[h-dim, seq] accumulated over dc.
                    for which, dst_t in (("q", qt), ("k", kt)):
                        cbase = 0 if which == "q" else FC
                        for sc in range(N_SC):
                            ps_t = [p1ps.tile([128, 512], F32, tag="mm",
                                              name=f"{which}ps{sc}_{i}")
                                    for i in range(HPC)]
                            for dc in range(N_DC):
                                d0 = dc * 128
                                xt_sb = p1sb.tile([128, 512], BF16, tag="xt",
                                                  bufs=12, name=f"x{which}{sc}_{dc}")
                                nc.sync.dma_start(
                                    xt_sb[:], xT[sc * D + d0:sc * D + d0 + 128, :])
                                w_sb = p1sb.tile([128, FC], BF16, tag="w",
                                                 bufs=6, name=f"w{which}{sc}_{dc}")
                                nc.scalar.dma_start(
                                    w_sb[:], wqk_t[d0:d0 + 128, cbase:cbase + FC])
                                for h in range(HPC):
                                    f0 = h * 128
                                    nc.tensor.matmul(ps_t[h][:],
                                                     w_sb[:, f0:f0 + 128],
                                                     xt_sb[:], start=(dc == 0),
                                                     stop=(dc == N_DC - 1))
                            for h in range(HPC):
                                rope_evict(ps_t[h], dst_t[h], sc,
                                           f"{which}{sc}_{h}")

                    # V pass: stationary = x sub-block, moving = wv row
                    # block; out[seq, feature] accumulated over dc.
                    for sc in range(N_SC):
                        v_ps = [p1ps.tile([128, 512], F32, tag="mm",
                                          name=f"vps{sc}_{i}") for i in range(4)]
                        for dc in range(N_DC):
                            d0 = dc * 128
                            xt_sb = p1sb.tile([128, 512], BF16, tag="xt",
                                              bufs=12, name=f"xv{sc}_{dc}")
                            nc.sync.dma_start(
                                xt_sb[:], xT[sc * D + d0:sc * D + d0 + 128, :])
                            wv_sb = p1sb.tile([128, FC], BF16, tag="w", bufs=6,
                                              name=f"wv{sc}_{dc}")
                            nc.scalar.dma_start(wv_sb[:], wv_t[d0:d0 + 128, :])
                            for sb in range(4):
                                nc.tensor.matmul(v_ps[sb][:],
                                                 xt_sb[:, sb * 128:(sb + 1) * 128],
                                                 wv_sb[:], start=(dc == 0),
                                                 stop=(dc == N_DC - 1))
                        for sb in range(4):
                            nc.vector.tensor_copy(vt[sc * 4 + sb][:], v_ps[sb][:])

                # wo tiles: pool opened before phase 2 so the loads overlap
                # attention compute (phase 4 consumes them).  Issued on
                # gpsimd, which is nearly idle in phase 2 (the scalar
                # engine runs the softmax Exps there).
                with tc.tile_pool(name="wop", bufs=1) as wop:
                    wo_sb = [wop.tile([128, FC], BF16, tag=f"wo{dc}",
                                      name=f"wo{dc}") for dc in range(N_DC)]
                    for dc in range(N_DC):
                        nc.gpsimd.dma_start(wo_sb[dc][:],
                                            wo_t[dc * 128:(dc + 1) * 128, :])

                    # -------- phase 2: attention, q-block outer so each
                    # 512-query chunk's AllGather can fire early
                    with tc.tile_pool(name="p2sb", bufs=1) as p2sb, \
                         tc.tile_pool(name="p2ps", bufs=2, space="PSUM") as p2ps, \
                         (tc.tile_pool(name="p2m", bufs=3) if mode == "generic" else
                          tc.tile_pool(name="p2m_unused", bufs=1)) as p2m:
                        if mode == "causal":
                            bm_sb = p2sb.tile([128, 4 * 512], F32, tag="bm")
                            nc.sync.dma_start(bm_sb[:], bmask[:])
                        for qb in range(N_SC):
                            q0 = qb * 512
                            kmax = 4 * qb + 3 if mode == "causal" else N_KC - 1
                            for h in range(HPC):
                                out_ps = p2ps.tile([128, 512], F32, tag="out",
                                                   name=f"o{qb}_{h}")
                                sums_ps = p2ps.tile([1, 512], F32, tag="sums",
                                                    name=f"s{qb}_{h}")

                                def issue_st(kc):
                                    k0 = kc * 128
                                    st_ps = p2ps.tile([128, 512], F32, tag="st",
                                                      bufs=4,
                                                      name=f"st{qb}_{h}_{kc}")
                                    nc.tensor.matmul(st_ps[:],
                                                     kt[h][:, k0:k0 + 128],
                                                     qt[h][:, q0:q0 + 512],
                                                     start=True, stop=True)
                                    e_sb = p2sb.tile([128, 512], BF16, tag="e",
                                                     bufs=6, name=f"e{qb}_{h}_{kc}")
                                    if mode == "generic":
                                        mt_sb = p2m.tile([128, 512], F32,
                                                         tag="mt", bufs=3,
                                                         name=f"mt{qb}_{h}_{kc}")
                                        nc.sync.dma_start(
                                            mt_sb[:],
                                            maskT[k0:k0 + 128, q0:q0 + 512])
                                        nc.vector.tensor_tensor(
                                            out=st_ps[:], in0=st_ps[:],
                                            in1=mt_sb[:],
                                            op=mybir.AluOpType.add)
                                        nc.scalar.activation(
                                            e_sb[:], st_ps[:],
                                            mybir.ActivationFunctionType.Exp,
                                            scale=float(SCALE))
                                    else:
                                        nc.scalar.activation(
                                            e_sb[:], st_ps[:],
                                            mybir.ActivationFunctionType.Exp,
                                            scale=float(SCALE))
                                        if mode == "causal":
                                            j = kc - 4 * qb
                                            if 0 <= j <= 3:
                                                nc.vector.tensor_tensor(
                                                    out=e_sb[:], in0=e_sb[:],
                                                    in1=bm_sb[:, j * 512:(j + 1) * 512],
                                                    op=mybir.AluOpType.mult)
                                    return e_sb

                                e_cur = issue_st(0)
                                for kc in range(kmax + 1):
                                    e_next = issue_st(kc + 1) if kc < kmax else None
                                    nc.tensor.matmul(
                                        out_ps[:],
                                        vt[kc][:, h * 128:(h + 1) * 128],
                                        e_cur[:], start=(kc == 0),
                                        stop=(kc == kmax))
                                    nc.tensor.matmul(sums_ps[:], ones_col[:],
                                                     e_cur[:], start=(kc == 0),
                                                     stop=(kc == kmax))
                                    e_cur = e_next
                                r_sb = p2sb.tile([1, 512], F32, tag="r", bufs=2,
                                                 name=f"r{qb}_{h}")
                                nc.vector.reciprocal_approx_fast(r_sb[:], sums_ps[:])
                                rb_sb = p2sb.tile([128, 512], F32, tag="rbs",
                                                  bufs=2, name=f"rbs{qb}_{h}")
                                nc.gpsimd.partition_broadcast(rb_sb[:], r_sb[:])
                                a_sb = p2sb.tile([128, 512], BF16, tag="a",
                                                 bufs=2, name=f"a{qb}_{h}")
                                nc.vector.tensor_tensor(out=a_sb[:],
                                                        in0=out_ps[:],
                                                        in1=rb_sb[:],
                                                        op=mybir.AluOpType.mult)
                                nc.sync.dma_start(
                                    cc_in[qb][h * 128:(h + 1) * 128, :], a_sb[:])
                            # -------- phase 3: chunked AllGather for this
                            # query block (overlaps later compute)
                            nc.gpsimd.collective_compute(
                                "AllGather",
                                mybir.AluOpType.bypass,
                                replica_groups=[list(range(N_CORES))],
                                ins=[cc_in[qb][:].opt()],
                                outs=[cc_out[qb][:].opt()],
                            )

                    # -------- phase 4: output projection column shard
                    with tc.tile_pool(name="p4sb", bufs=1) as p4sb, \
                         tc.tile_pool(name="p4ps", bufs=8, space="PSUM") as p4ps:
                        for sc in range(N_SC):
                            s0 = sc * 512
                            o_ps = [p4ps.tile([128, 512], F32, tag="o",
                                              name=f"ops{sc}_{i}")
                                    for i in range(4)]
                            for dc in range(N_DC):
                                d0 = dc * 128
                                at_sb = p4sb.tile([128, 512], BF16, tag="at",
                                                  bufs=8, name=f"at{sc}_{dc}")
                                aeng = nc.sync if dc % 2 == 0 else nc.scalar
                                aeng.dma_start(at_sb[:],
                                               cc_out[sc][d0:d0 + 128, :])
                                for sb in range(4):
                                    nc.tensor.matmul(
                                        o_ps[sb][:],
                                        at_sb[:, sb * 128:(sb + 1) * 128],
                                        wo_sb[dc][:], start=(dc == 0),
                                        stop=(dc == N_DC - 1))
                            for sb in range(4):
                                o_sb = p4sb.tile([128, FC], F32, tag="osb",
                                                 bufs=2, name=f"osb{sc}_{sb}")
                                nc.vector.tensor_copy(o_sb[:], o_ps[sb][:])
                                r0 = s0 + sb * 128
                                nc.scalar.dma_start(out_t[r0:r0 + 128, :],
                                                    o_sb[:])

    nc.finalize()
    return nc


_MODULE_CACHE: dict = {}


def _get_module(mode: str):
    if mode not in _MODULE_CACHE:
        _MODULE_CACHE[mode] = _build_module(mode)
    return _MODULE_CACHE[mode]


def _rope_perm() -> np.ndarray:
    """Per-head permutation: interleaved pairs (2j, 2j+1) -> (j, j+64)."""
    p = np.empty(HD, dtype=np.int64)
    p[0:64] = np.arange(0, HD, 2)
    p[64:128] = np.arange(1, HD, 2)
    full = np.concatenate([h * HD + p for h in range(HPC)])
    return full


def _canonical_causal_mask() -> np.ndarray:
    m = np.where(np.tril(np.ones((S, S), dtype=bool)), np.float32(0.0),
                 np.float32(NEG))
    return m.astype(np.float32)


def _numpy_fallback(x, freqs_cos, freqs_sin, mask, input_indexes, cache_k,
                    cache_v, wq, wk, wv, wo):
    """Exact reference reimplementation (host, fp32). Only used for inputs
    the device kernel does not model (non-arange cache indexes)."""
    B = x.shape[0]
    xf = x.astype(np.float32)

    def rope(t):
        tr = t[..., 0::2]
        ti = t[..., 1::2]
        c = freqs_cos[None, :, None, :]
        s = freqs_sin[None, :, None, :]
        outr = tr * c - ti * s
        outi = tr * s + ti * c
        return np.stack([outr, outi], axis=-1).reshape(t.shape)

    xq = (xf @ wq.T).reshape(B, S, N_HEADS, HD)
    xk = (xf @ wk.T).reshape(B, S, N_HEADS, HD)
    xv = (xf @ wv.T).reshape(B, S, N_HEADS, HD)
    xq = rope(xq)
    xk = rope(xk)
    keys = np.array(cache_k)
    vals = np.array(cache_v)
    keys[:, input_indexes] = xk
    vals[:, input_indexes] = xv
    scores = np.einsum("bqhd,bkhd->bhqk", xq, keys) / np.sqrt(HD)
    scores = scores + mask
    scores = scores - scores.max(axis=-1, keepdims=True)
    e = np.exp(scores)
    probs = e / e.sum(axis=-1, keepdims=True)
    out = np.einsum("bhqk,bkhd->bqhd", probs, vals)
    return (out.reshape(B, S, N_HEADS * HD) @ wo.T).astype(np.float32)


def _prepare_in_maps(x, freqs_cos, freqs_sin, mask, wq, wk, wv, wo, mode):
    x2 = np.ascontiguousarray(x.reshape(S, D), dtype=np.float32)
    xTf = x2.T  # [D, S]
    xT = to_bf16(np.concatenate(
        [xTf[:, sc * 512:(sc + 1) * 512] for sc in range(N_SC)], axis=0))

    cosb = np.empty((128, S), dtype=np.float32)
    sinb = np.empty((128, S), dtype=np.float32)
    fc = np.asarray(freqs_cos, dtype=np.float32).T  # [64, S]
    fs = np.asarray(freqs_sin, dtype=np.float32).T
    cosb[0:64] = fc
    cosb[64:128] = fc
    sinb[0:64] = fs
    sinb[64:128] = fs

    perm = _rope_perm()

    if mode == "causal":
        kl = np.arange(128, dtype=np.int64)[:, None]
        ql = np.arange(512, dtype=np.int64)[None, :]
        bmask = np.concatenate(
            [(kl <= ql - 128 * j).astype(np.float32) for j in range(4)], axis=1)
        bmask = np.ascontiguousarray(bmask)
    elif mode == "generic":
        maskT = np.ascontiguousarray(
            (np.asarray(mask, dtype=np.float32)[0, 0].T) / np.float32(SCALE))

    in_maps = []
    for c in range(N_CORES):
        r0 = c * FC
        wq_c = np.asarray(wq[r0:r0 + FC], dtype=np.float32)[perm]
        wk_c = np.asarray(wk[r0:r0 + FC], dtype=np.float32)[perm]
        wqk_c = np.concatenate([wq_c, wk_c], axis=0)      # [1024, D]
        wv_c = np.asarray(wv[r0:r0 + FC], dtype=np.float32)
        wo_c = np.asarray(wo[r0:r0 + FC], dtype=np.float32)
        m = {
            "xT": xT,
            "wqk_t": to_bf16(wqk_c.T),
            "wv_t": to_bf16(wv_c.T),
            "wo_t": to_bf16(wo_c.T),
            "cosb": cosb,
            "sinb": sinb,
        }
        if mode == "causal":
            m["bmask"] = bmask
        elif mode == "generic":
            m["maskT"] = maskT
        in_maps.append(m)
    return in_maps


def _run(inputs: dict, trace: bool = False):
    x = np.asarray(inputs["x"])
    freqs_cos = np.asarray(inputs["freqs_cos"])
    freqs_sin = np.asarray(inputs["freqs_sin"])
    mask = np.asarray(inputs["mask"], dtype=np.float32)
    input_indexes = np.asarray(inputs["input_indexes"])
    wq = np.asarray(inputs["wq"])
    wk = np.asarray(inputs["wk"])
    wv = np.asarray(inputs["wv"])
    wo = np.asarray(inputs["wo"])

    if not np.array_equal(input_indexes.astype(np.int64), np.arange(S)):
        out = _numpy_fallback(x, freqs_cos, freqs_sin, mask, input_indexes,
                              inputs["cache_k"], inputs["cache_v"], wq, wk, wv, wo)
        return out, None

    m2 = mask[0, 0]
    if not m2.any():
        mode = "zero"
    elif np.array_equal(m2, _canonical_causal_mask()):
        mode = "causal"
    else:
        mode = "generic"
    nc = _get_module(mode)
    in_maps = _prepare_in_maps(x, freqs_cos, freqs_sin, mask, wq, wk, wv, wo,
                               mode)
    res = run_bass_kernel_spmd(nc, in_maps, core_ids=list(range(N_CORES)),
                               trace=trace)
    out = np.concatenate([res.results[c]["out"] for c in range(N_CORES)],
                         axis=1)
    return out.reshape(1, S, D).astype(np.float32), res


def kernel(**inputs) -> np.ndarray:
    out, _ = _run(inputs, trace=False)
    return out


# revision 11
# speedup vs baseline: 1.0270x; 1.0270x over previous
"""Trainium2 Bass kernel for a 32-head causal attention layer.

Problem: B=1, S=2048, D=4096, 32 heads x 128 head-dim, fp32, llama-style
interleaved RoPE on q/k, KV-cache index_copy (identity for arange indexes),
additive mask + softmax, output projection.

Sharding (8 NeuronCores, tensor-parallel by heads):
  - core c owns heads [4c, 4c+4): wq/wk/wv output rows [512c, 512c+512)
  - per-core: QKV projections -> RoPE -> attention -> attn.T shard [512, 2048]
  - 4 chunked on-device AllGathers (one per 512-query block) overlap the
    collective with attention/output-projection compute
  - core c computes output column shard out[:, 512c:512c+512] = attn @ wo_c.T
  - host concatenates the 8 column shards (pure unshard, no arithmetic)

All matmul operands are bf16 (RNE-rounded on host for the inputs; on-device
producers write bf16-dtype tiles which round on writeback).  PSUM
accumulation is fp32 throughout.  bf16 halves DMA/SBUF/collective traffic,
which is what keeps the PE array fed (fp32 operand streaming saturated the
DMA engines and stalled the PE at every phase boundary).

Mask modes (selected on host per input):
  - "zero": mask is all zeros (full bidirectional attention) -- no mask
    tensor on device at all.
  - "causal": canonical tril mask -- per-128-key-block multiplicative mask
    on just the diagonal blocks; off-diagonal key blocks are skipped.
  - "generic": arbitrary additive mask streamed from DRAM.

RoPE trick: weight rows of wq/wk are permuted per head on the host so the
interleaved pairs (2j, 2j+1) become (j, j+64).  Scores are invariant under
a per-head orthogonal permutation applied to both q and k, and the rotation
then only needs partition-range [0:64]/[64:128] cross-multiplies, which map
to plain DVE tensor_tensor ops (no strided partition access).

The 1/sqrt(128) score scale is folded into the Exp activation's scale
operand.  Softmax runs over the partition (key) axis: scores are computed
transposed st[k, q] = K Q^T, summed with a ones-vector matmul, and
normalized after the PV matmul via reciprocal_approx_fast + outer-product
broadcast.
"""

import numpy as np
import ml_dtypes

import concourse.bass as bass
import concourse.mybir as mybir
import concourse.tile as tile
from concourse import bacc
from concourse.bass_utils import run_bass_kernel_spmd

F32 = mybir.dt.float32
BF16 = mybir.dt.bfloat16

S = 2048
D = 4096
HD = 128
N_HEADS = 32
N_CORES = 8
HPC = N_HEADS // N_CORES          # heads per core = 4
FC = HPC * HD                     # features per core = 512
N_DC = D // 128                   # 32 contraction chunks
N_SC = S // 512                   # 4 seq chunks of 512
N_KC = S // 128                   # 16 key chunks of 128
SCALE = 1.0 / np.sqrt(HD)
NEG = -1e9


def to_bf16(a: np.ndarray) -> np.ndarray:
    return np.ascontiguousarray(a, dtype=np.float32).astype(ml_dtypes.bfloat16)


def _build_module(mode: str):
    assert mode in ("zero", "causal", "generic")
    nc = bacc.Bacc(num_devices=N_CORES)

    xT = nc.dram_tensor("xT", [2 * D, 1024], BF16, kind="ExternalInput")
    wqk_t = nc.dram_tensor("wqk_t", [D, 2 * FC], BF16, kind="ExternalInput")
    wv_t = nc.dram_tensor("wv_t", [D, FC], BF16, kind="ExternalInput")
    wo_t = nc.dram_tensor("wo_t", [D, FC], BF16, kind="ExternalInput")
    cosb = nc.dram_tensor("cosb", [128, S], F32, kind="ExternalInput")
    sinb = nc.dram_tensor("sinb", [128, S], F32, kind="ExternalInput")
    if mode == "causal":
        bmask = nc.dram_tensor("bmask", [128, 4 * 512], F32, kind="ExternalInput")
    elif mode == "generic":
        maskT = nc.dram_tensor("maskT", [S, S], F32, kind="ExternalInput")
    out_t = nc.dram_tensor("out", [S, FC], F32, kind="ExternalOutput")

    with tile.TileContext(nc) as tc:
        with tc.tile_pool(name="const", bufs=1) as constp, \
             tc.tile_pool(name="dram", bufs=1, space="DRAM") as dram:
            cc_in = [dram.tile([FC, 512], BF16, name=f"cc_in{i}")
                     for i in range(N_SC)]
            cc_out = [dram.tile([D, 512], BF16, addr_space="Shared",
                                name=f"cc_out{i}") for i in range(N_SC)]

            ones_f = constp.tile([128, 1], F32, tag="ones_f")
            nc.vector.memset(ones_f[:], 1.0)
            ones_col = constp.tile([128, 1], BF16, tag="ones_col")
            nc.vector.tensor_copy(ones_col[:], ones_f[:])

            with tc.tile_pool(name="qkv", bufs=1) as qkvp:
                qt = [qkvp.tile([128, S], BF16, tag=f"qt{h}", name=f"qt{h}")
                      for h in range(HPC)]
                kt = [qkvp.tile([128, S], BF16, tag=f"kt{h}", name=f"kt{h}")
                      for h in range(HPC)]
                vt = [qkvp.tile([128, FC], BF16, tag=f"vt{b}", name=f"vt{b}")
                      for b in range(N_KC)]

                # -------- phase 1: QKV projections + rope.  Two seq-pair
                # groups (columns 0:1024 and 1024:2048 of x^T); per group
                # the x tiles ([128,1024] bf16, 2KB DMA lines) and the
                # weight tiles are group-resident in SBUF.  Q/K/V passes
                # sweep one head (two 512-col accumulators) at a time over
                # all 32 contraction chunks, so at most 4 PSUM banks are
                # live per head-pass and the rope eviction of head h runs
                # on the DVE entirely under head h+1's matmuls.  The
                # stationary weight block is reused across the seq pair
                # (half the LDWEIGHTS).
                with tc.tile_pool(name="p1sb", bufs=1) as p1sb, \
                     tc.tile_pool(name="p1ps", bufs=8, space="PSUM") as p1ps:
                    cos_sb = p1sb.tile([128, S], F32, tag="cos")
                    sin_sb = p1sb.tile([128, S], F32, tag="sin")
                    nc.sync.dma_start(cos_sb[:], cosb[:])
                    nc.sync.dma_start(sin_sb[:], sinb[:])

                    def rope_evict(ps, dst, sc, tag):
                        # dst[0:64]  = ps[0:64]*c - ps[64:128]*s
                        # dst[64:128]= ps[0:64]*s + ps[64:128]*c
                        # (cross-partition reads only on the PSUM operand:
                        # both-SBUF operands need equal base partitions per
                        # the BIR verifier; cos rows are duplicated so the
                        # cos multiply is one full-width op)
                        s0 = sc * 512
                        cs = cos_sb[:, s0:s0 + 512]
                        sn = sin_sb[:, s0:s0 + 512]
                        t1 = p1sb.tile([128, 512], F32, tag="t1", bufs=2,
                                       name=f"t1_{tag}")
                        t2 = p1sb.tile([128, 512], F32, tag="t2", bufs=2,
                                       name=f"t2_{tag}")
                        t3 = p1sb.tile([128, 512], F32, tag="t3", bufs=2,
                                       name=f"t3_{tag}")
                        mul = mybir.AluOpType.mult
                        nc.vector.tensor_tensor(out=t1[:], in0=ps[:],
                                                in1=cs, op=mul)
                        nc.vector.tensor_tensor(out=t2[0:64, :], in0=ps[64:128, :],
                                                in1=sn[0:64, :], op=mul)
                        nc.vector.tensor_tensor(out=t3[64:128, :], in0=ps[0:64, :],
                                                in1=sn[64:128, :], op=mul)
                        nc.vector.tensor_tensor(out=dst[0:64, s0:s0 + 512],
                                                in0=t1[0:64, :], in1=t2[0:64, :],
                                                op=mybir.AluOpType.subtract)
                        nc.vector.tensor_tensor(out=dst[64:128, s0:s0 + 512],
                                                in0=t3[64:128, :], in1=t1[64:128, :],
                                                op=mybir.AluOpType.add)

                    for g in range(2):
                        # group-resident x: 32 tiles [128,1024] bf16 = 64KB
                        # per partition (xT rows g*D..g*D+D = x^T columns
                        # g*1024..g*1024+1024)
                        xts = []
                        for dc in range(N_DC):
                            d0 = dc * 128
                            xt_sb = p1sb.tile([128, 1024], BF16, tag="xt",
                                              bufs=33, name=f"x{g}_{dc}")
                            xeng = nc.sync if dc % 2 == 0 else nc.scalar
                            xeng.dma_start(xt_sb[:],
                                           xT[g * D + d0:g * D + d0 + 128, :])
                            xts.append(xt_sb)

                        # Q then K, in head-pair passes: stationary = one
                        # head's weight block, reused across the seq pair.
                        for which, dst_t, cbase in (("q", qt, 0),
                                                    ("k", kt, FC)):
                            for hp in range(2):
                                c0 = cbase + hp * 256
                                ps_t = [p1ps.tile([128, 512], F32, tag="mm",
                                                  name=f"{which}ps{g}_{hp}_{j}")
                                        for j in range(4)]
                                for dc in range(N_DC):
                                    d0 = dc * 128
                                    w_sb = p1sb.tile([128, 256], BF16,
                                                     tag="wp", bufs=8,
                                                     name=f"w{which}{g}_{hp}_{dc}")
                                    weng = nc.scalar if dc % 2 == 0 else nc.sync
                                    weng.dma_start(
                                        w_sb[:], wqk_t[d0:d0 + 128, c0:c0 + 256])
                                    for hh in range(2):
                                        for s in range(2):
                                            nc.tensor.matmul(
                                                ps_t[hh * 2 + s][:],
                                                w_sb[:, hh * 128:(hh + 1) * 128],
                                                xts[dc][:, s * 512:(s + 1) * 512],
                                                start=(dc == 0),
                                                stop=(dc == N_DC - 1))
                                for hh in range(2):
                                    for s in range(2):
                                        rope_evict(ps_t[hh * 2 + s],
                                                   dst_t[2 * hp + hh], 2 * g + s,
                                                   f"{which}{g}_{hp}_{hh}_{s}")

                        # V: one 8-bank pass; stationary = x seq sub-block,
                        # moving = wv row block (streamed once per group).
                        v_ps = [p1ps.tile([128, 512], F32, tag="mm",
                                          name=f"vps{g}_{i}") for i in range(8)]
                        for dc in range(N_DC):
                            d0 = dc * 128
                            wv_sb = p1sb.tile([128, FC], BF16, tag="wv", bufs=6,
                                              name=f"wv{g}_{dc}")
                            veng = nc.scalar if dc % 2 == 0 else nc.sync
                            veng.dma_start(wv_sb[:], wv_t[d0:d0 + 128, :])
                            for sb in range(8):
                                nc.tensor.matmul(
                                    v_ps[sb][:],
                                    xts[dc][:, sb * 128:(sb + 1) * 128],
                                    wv_sb[:], start=(dc == 0),
                                    stop=(dc == N_DC - 1))
                        for sb in range(8):
                            nc.vector.tensor_copy(vt[g * 8 + sb][:],
                                                  v_ps[sb][:])

                # wo tiles: pool opened before phase 2 so the loads overlap
                # attention compute (phase 4 consumes them).  Issued on
                # gpsimd, which is nearly idle in phase 2 (the scalar
                # engine runs the softmax Exps there).
                with tc.tile_pool(name="wop", bufs=1) as wop:
                    wo_sb = [wop.tile([128, FC], BF16, tag=f"wo{dc}",
                                      name=f"wo{dc}") for dc in range(N_DC)]
                    for dc in range(N_DC):
                        nc.gpsimd.dma_start(wo_sb[dc][:],
                                            wo_t[dc * 128:(dc + 1) * 128, :])

                    # -------- phase 2: attention, q-block outer so each
                    # 512-query chunk's AllGather can fire early
                    with tc.tile_pool(name="p2sb", bufs=1) as p2sb, \
                         tc.tile_pool(name="p2ps", bufs=2, space="PSUM") as p2ps, \
                         (tc.tile_pool(name="p2m", bufs=3) if mode == "generic" else
                          tc.tile_pool(name="p2m_unused", bufs=1)) as p2m:
                        if mode == "causal":
                            bm_sb = p2sb.tile([128, 4 * 512], F32, tag="bm")
                            nc.sync.dma_start(bm_sb[:], bmask[:])
                        for qb in range(N_SC):
                            q0 = qb * 512
                            kmax = 4 * qb + 3 if mode == "causal" else N_KC - 1
                            for h in range(HPC):
                                out_ps = p2ps.tile([128, 512], F32, tag="out",
                                                   name=f"o{qb}_{h}")
                                sums_ps = p2ps.tile([1, 512], F32, tag="sums",
                                                    name=f"s{qb}_{h}")

                                def issue_st(kc):
                                    k0 = kc * 128
                                    st_ps = p2ps.tile([128, 512], F32, tag="st",
                                                      bufs=3,
                                                      name=f"st{qb}_{h}_{kc}")
                                    nc.tensor.matmul(st_ps[:],
                                                     kt[h][:, k0:k0 + 128],
                                                     qt[h][:, q0:q0 + 512],
                                                     start=True, stop=True)
                                    e_sb = p2sb.tile([128, 512], BF16, tag="e",
                                                     bufs=6, name=f"e{qb}_{h}_{kc}")
                                    if mode == "generic":
                                        mt_sb = p2m.tile([128, 512], F32,
                                                         tag="mt", bufs=3,
                                                         name=f"mt{qb}_{h}_{kc}")
                                        nc.sync.dma_start(
                                            mt_sb[:],
                                            maskT[k0:k0 + 128, q0:q0 + 512])
                                        nc.vector.tensor_tensor(
                                            out=st_ps[:], in0=st_ps[:],
                                            in1=mt_sb[:],
                                            op=mybir.AluOpType.add)
                                        nc.scalar.activation(
                                            e_sb[:], st_ps[:],
                                            mybir.ActivationFunctionType.Exp,
                                            scale=float(SCALE))
                                    else:
                                        nc.scalar.activation(
                                            e_sb[:], st_ps[:],
                                            mybir.ActivationFunctionType.Exp,
                                            scale=float(SCALE))
                                        if mode == "causal":
                                            j = kc - 4 * qb
                                            if 0 <= j <= 3:
                                                nc.vector.tensor_tensor(
                                                    out=e_sb[:], in0=e_sb[:],
                                                    in1=bm_sb[:, j * 512:(j + 1) * 512],
                                                    op=mybir.AluOpType.mult)
                                    return e_sb

                                e_cur = issue_st(0)
                                for kc in range(kmax + 1):
                                    e_next = issue_st(kc + 1) if kc < kmax else None
                                    nc.tensor.matmul(
                                        out_ps[:],
                                        vt[kc][:, h * 128:(h + 1) * 128],
                                        e_cur[:], start=(kc == 0),
                                        stop=(kc == kmax))
                                    nc.tensor.matmul(sums_ps[:], ones_col[:],
                                                     e_cur[:], start=(kc == 0),
                                                     stop=(kc == kmax))
                                    e_cur = e_next
                                r_sb = p2sb.tile([1, 512], F32, tag="r", bufs=2,
                                                 name=f"r{qb}_{h}")
                                nc.vector.reciprocal_approx_fast(r_sb[:], sums_ps[:])
                                rb_sb = p2sb.tile([128, 512], F32, tag="rbs",
                                                  bufs=2, name=f"rbs{qb}_{h}")
                                nc.gpsimd.partition_broadcast(rb_sb[:], r_sb[:])
                                a_sb = p2sb.tile([128, 512], BF16, tag="a",
                                                 bufs=2, name=f"a{qb}_{h}")
                                nc.vector.tensor_tensor(out=a_sb[:],
                                                        in0=out_ps[:],
                                                        in1=rb_sb[:],
                                                        op=mybir.AluOpType.mult)
                                nc.sync.dma_start(
                                    cc_in[qb][h * 128:(h + 1) * 128, :], a_sb[:])
                            # -------- phase 3: chunked AllGather for this
                            # query block (overlaps later compute)
                            nc.gpsimd.collective_compute(
                                "AllGather",
                                mybir.AluOpType.bypass,
                                replica_groups=[list(range(N_CORES))],
                                ins=[cc_in[qb][:].opt()],
                                outs=[cc_out[qb][:].opt()],
                            )

                    # -------- phase 4: output projection column shard
                    with tc.tile_pool(name="p4sb", bufs=1) as p4sb, \
                         tc.tile_pool(name="p4ps", bufs=8, space="PSUM") as p4ps:
                        for sc in range(N_SC):
                            s0 = sc * 512
                            o_ps = [p4ps.tile([128, 512], F32, tag="o",
                                              name=f"ops{sc}_{i}")
                                    for i in range(4)]
                            for dc in range(N_DC):
                                d0 = dc * 128
                                at_sb = p4sb.tile([128, 512], BF16, tag="at",
                                                  bufs=8, name=f"at{sc}_{dc}")
                                aeng = nc.sync if dc % 2 == 0 else nc.scalar
                                aeng.dma_start(at_sb[:],
                                               cc_out[sc][d0:d0 + 128, :])
                                for sb in range(4):
                                    nc.tensor.matmul(
                                        o_ps[sb][:],
                                        at_sb[:, sb * 128:(sb + 1) * 128],
                                        wo_sb[dc][:], start=(dc == 0),
                                        stop=(dc == N_DC - 1))
                            for sb in range(4):
                                o_sb = p4sb.tile([128, FC], F32, tag="osb",
                                                 bufs=2, name=f"osb{sc}_{sb}")
                                nc.vector.tensor_copy(o_sb[:], o_ps[sb][:])
                                r0 = s0 + sb * 128
                                nc.scalar.dma_start(out_t[r0:r0 + 128, :],
                                                    o_sb[:])

    nc.finalize()
    return nc


_MODULE_CACHE: dict = {}


def _get_module(mode: str):
    if mode not in _MODULE_CACHE:
        _MODULE_CACHE[mode] = _build_module(mode)
    return _MODULE_CACHE[mode]


def _rope_perm() -> np.ndarray:
    """Per-head permutation: interleaved pairs (2j, 2j+1) -> (j, j+64)."""
    p = np.empty(HD, dtype=np.int64)
    p[0:64] = np.arange(0, HD, 2)
    p[64:128] = np.arange(1, HD, 2)
    full = np.concatenate([h * HD + p for h in range(HPC)])
    return full


def _canonical_causal_mask() -> np.ndarray:
    m = np.where(np.tril(np.ones((S, S), dtype=bool)), np.float32(0.0),
                 np.float32(NEG))
    return m.astype(np.float32)


def _numpy_fallback(x, freqs_cos, freqs_sin, mask, input_indexes, cache_k,
                    cache_v, wq, wk, wv, wo):
    """Exact reference reimplementation (host, fp32). Only used for inputs
    the device kernel does not model (non-arange cache indexes)."""
    B = x.shape[0]
    xf = x.astype(np.float32)

    def rope(t):
        tr = t[..., 0::2]
        ti = t[..., 1::2]
        c = freqs_cos[None, :, None, :]
        s = freqs_sin[None, :, None, :]
        outr = tr * c - ti * s
        outi = tr * s + ti * c
        return np.stack([outr, outi], axis=-1).reshape(t.shape)

    xq = (xf @ wq.T).reshape(B, S, N_HEADS, HD)
    xk = (xf @ wk.T).reshape(B, S, N_HEADS, HD)
    xv = (xf @ wv.T).reshape(B, S, N_HEADS, HD)
    xq = rope(xq)
    xk = rope(xk)
    keys = np.array(cache_k)
    vals = np.array(cache_v)
    keys[:, input_indexes] = xk
    vals[:, input_indexes] = xv
    scores = np.einsum("bqhd,bkhd->bhqk", xq, keys) / np.sqrt(HD)
    scores = scores + mask
    scores = scores - scores.max(axis=-1, keepdims=True)
    e = np.exp(scores)
    probs = e / e.sum(axis=-1, keepdims=True)
    out = np.einsum("bhqk,bkhd->bqhd", probs, vals)
    return (out.reshape(B, S, N_HEADS * HD) @ wo.T).astype(np.float32)


def _prepare_in_maps(x, freqs_cos, freqs_sin, mask, wq, wk, wv, wo, mode):
    x2 = np.ascontiguousarray(x.reshape(S, D), dtype=np.float32)
    xTf = x2.T  # [D, S]
    xT = to_bf16(np.concatenate(
        [xTf[:, g * 1024:(g + 1) * 1024] for g in range(2)], axis=0))

    cosb = np.empty((128, S), dtype=np.float32)
    sinb = np.empty((128, S), dtype=np.float32)
    fc = np.asarray(freqs_cos, dtype=np.float32).T  # [64, S]
    fs = np.asarray(freqs_sin, dtype=np.float32).T
    cosb[0:64] = fc
    cosb[64:128] = fc
    sinb[0:64] = fs
    sinb[64:128] = fs

    perm = _rope_perm()

    if mode == "causal":
        kl = np.arange(128, dtype=np.int64)[:, None]
        ql = np.arange(512, dtype=np.int64)[None, :]
        bmask = np.concatenate(
            [(kl <= ql - 128 * j).astype(np.float32) for j in range(4)], axis=1)
        bmask = np.ascontiguousarray(bmask)
    elif mode == "generic":
        maskT = np.ascontiguousarray(
            (np.asarray(mask, dtype=np.float32)[0, 0].T) / np.float32(SCALE))

    in_maps = []
    for c in range(N_CORES):
        r0 = c * FC
        wq_c = np.asarray(wq[r0:r0 + FC], dtype=np.float32)[perm]
        wk_c = np.asarray(wk[r0:r0 + FC], dtype=np.float32)[perm]
        wqk_c = np.concatenate([wq_c, wk_c], axis=0)      # [1024, D]
        wv_c = np.asarray(wv[r0:r0 + FC], dtype=np.float32)
        wo_c = np.asarray(wo[r0:r0 + FC], dtype=np.float32)
        m = {
            "xT": xT,
            "wqk_t": to_bf16(wqk_c.T),
            "wv_t": to_bf16(wv_c.T),
            "wo_t": to_bf16(wo_c.T),
            "cosb": cosb,
            "sinb": sinb,
        }
        if mode == "causal":
            m["bmask"] = bmask
        elif mode == "generic":
            m["maskT"] = maskT
        in_maps.append(m)
    return in_maps


def _run(inputs: dict, trace: bool = False):
    x = np.asarray(inputs["x"])
    freqs_cos = np.asarray(inputs["freqs_cos"])
    freqs_sin = np.asarray(inputs["freqs_sin"])
    mask = np.asarray(inputs["mask"], dtype=np.float32)
    input_indexes = np.asarray(inputs["input_indexes"])
    wq = np.asarray(inputs["wq"])
    wk = np.asarray(inputs["wk"])
    wv = np.asarray(inputs["wv"])
    wo = np.asarray(inputs["wo"])

    if not np.array_equal(input_indexes.astype(np.int64), np.arange(S)):
        out = _numpy_fallback(x, freqs_cos, freqs_sin, mask, input_indexes,
                              inputs["cache_k"], inputs["cache_v"], wq, wk, wv, wo)
        return out, None

    m2 = mask[0, 0]
    if not m2.any():
        mode = "zero"
    elif np.array_equal(m2, _canonical_causal_mask()):
        mode = "causal"
    else:
        mode = "generic"
    nc = _get_module(mode)
    in_maps = _prepare_in_maps(x, freqs_cos, freqs_sin, mask, wq, wk, wv, wo,
                               mode)
    res = run_bass_kernel_spmd(nc, in_maps, core_ids=list(range(N_CORES)),
                               trace=trace)
    out = np.concatenate([res.results[c]["out"] for c in range(N_CORES)],
                         axis=1)
    return out.reshape(1, S, D).astype(np.float32), res


def kernel(**inputs) -> np.ndarray:
    out, _ = _run(inputs, trace=False)
    return out
